# revision 19
# baseline (speedup 1.0000x reference)
"""Trainium2 Bass kernel for nn_Attention_70291434766394.

GQA attention: B=2, T=2048, D=2048, H=16 heads, KV=4 kv-heads, HD=128,
RMSNorm on q/k, interleaved RoPE, causal mask, f32 reference.

Sharding (8 NeuronCores): 2 batch groups x 4 tensor-parallel ranks.
Core c: batch b=c//4, rank r=c%4 -> q heads [4r,4r+4), kv head r.
Per core: QKV projections in transposed layout, flash attention with
S^T-layout softmax (partition-axis denominators via PE matmuls, no
transposes in the hot loop), AllGather (split in two head-pair halves
per block) of normalized per-head outputs within each 4-rank group,
column-sharded output projection. The host only slices/relayouts
inputs and concatenates the output shards.

v1 scheduling/precision changes vs v0 baseline:
  - all device inputs host-prearranged to partition-major contiguous
    layouts (cheap DMA descriptors), x/tables/q/k path in bf16
  - startup: x block0 + tables first on the sync queue, weights chunked,
    cold constants issued on the scalar queue
  - PSUM replanned (qp/vn/fin share 2 banks, ot 4, sps/ssq/kssq 2) so
    the Wo matmuls of block j-1 are not bank-blocked behind attn(j);
    softmax denominators accumulate per-g on the DVE
  - broadcasts via gpsimd.partition_broadcast instead of PE matmuls
  - reciprocals via DVE reciprocal_approx_fast (~18 bits)
  - AllGather split into two per-block halves to shorten the tail
"""
import sys

for _p in ("/opt/trn_rl_repo", "/root/.axon_site/_ro/trn_rl_repo"):
    if _p not in sys.path:
        sys.path.insert(0, _p)

from concourse import bass_utils

import numpy as np
import concourse.bass as bass
import concourse.mybir as mybir
import concourse.tile as tile
from concourse import bacc

F32 = mybir.dt.float32
F32R = mybir.dt.float32r
BF16 = mybir.dt.bfloat16
AF = mybir.ActivationFunctionType
OP = mybir.AluOpType

B, T, D = 2, 2048, 2048
H, KV, HD = 16, 4, 128
EPS = 1e-6
NB = 4
TQB = 512
NK = D // 128
GROUPS = [[0, 1, 2, 3], [4, 5, 6, 7]]
N_CORES = 8
# global c16 indices covered by each AllGather half (heads 0,1 / 2,3 per rank)
C16_HALF = [[4 * r + h for r in range(4) for h in (0, 1)],
            [4 * r + h for r in range(4) for h in (2, 3)]]


def build(mm_fast=True, p_dt_bf16=True, g_dt_bf16=True, qk_bf16=True,
          causal=True, single=False, bcast_gpsimd=True):
    """qk_bf16: x/weights/q^T/k^T/rope tables in bf16 (else f32r).
    p_dt_bf16 / g_dt_bf16 kept for interface compat (always bf16 here)."""
    XW = BF16 if qk_bf16 else F32R    # x, Wq/Wk/Wv
    QKD = BF16 if qk_bf16 else F32R   # roped q^T/k^T, rope tables
    MMD = F32R if mm_fast else F32
    PDT = BF16                        # softmaxed P, v, E
    GDT = BF16                        # gather path (o_norm, AG, og, Wo)

    nc = bacc.Bacc("TRN2", target_bir_lowering=False, debug=False,
                   num_devices=1 if single else N_CORES)
    import contextlib
    lp = nc.allow_low_precision(reason="bf16/f32r matmul operand rounding")

    def inp(name, shape, dt=F32):
        return nc.dram_tensor(name, list(shape), dt, kind="ExternalInput").ap()

    xp = inp("xp", [128, NB, NK, TQB], XW)
    wq = inp("wq", [128, NK, 4 * HD], XW)
    wk = inp("wk", [128, NK, HD], XW)
    wv = inp("wv", [128, NK, HD], XW)
    wo = inp("wo", [128, NK, TQB], GDT)
    tabs = inp("tabs", [128, NB, 4, TQB], QKD)   # cq, sq, ck, sk per block
    pat = inp("pat", [128, 4, TQB], PDT)
    E4 = inp("E4", [128, 4, 4], PDT)
    E4f = inp("E4f", [128, 4, 4], MMD)
    sel4 = inp("sel4", [4, 4, 128], MMD)         # only used if not bcast_gpsimd
    onesc = inp("onesc", [128, 1], PDT)
    eye = inp("eye", [128, 128], PDT)
    out = nc.dram_tensor("out", [T, TQB], F32, kind="ExternalOutput").ap()

    with lp, tile.TileContext(nc) as tc:
        with tc.tile_pool(name="const", bufs=1) as cpool, \
             tc.tile_pool(name="kv", bufs=1) as kvpool, \
             tc.tile_pool(name="xt", bufs=2) as xtpool, \
             tc.tile_pool(name="tbl", bufs=2) as tblpool, \
             tc.tile_pool(name="qt", bufs=2) as qtpool, \
             tc.tile_pool(name="p", bufs=6) as ppool, \
             tc.tile_pool(name="wk1", bufs=2) as wpool, \
             tc.tile_pool(name="wk2", bufs=4) as w2pool, \
             tc.tile_pool(name="og", bufs=8) as ogpool, \
             tc.tile_pool(name="sm", bufs=2) as smpool, \
             tc.tile_pool(name="psA", bufs=2, space="PSUM") as psA, \
             tc.tile_pool(name="psOT", bufs=4, space="PSUM") as psOT, \
             tc.tile_pool(name="psC", bufs=2, space="PSUM") as psC, \
             tc.tile_pool(name="dram", bufs=3, space="DRAM") as dpool:

            # ---- hot-path loads first (sync queue) ----
            def load_block(j):
                xt = xtpool.tile([128, NK, TQB], XW, name=f"xt{j}", tag="xt")
                for c in range(4):
                    nc.sync.dma_start(xt[:, 4 * c:4 * (c + 1), :],
                                      xp[:, j, 4 * c:4 * (c + 1), :])
                return xt

            def load_tab(j):
                tab = tblpool.tile([128, 4, TQB], QKD, name=f"tab{j}",
                                   tag="tab")
                nc.sync.dma_start(tab[:], tabs[:, j])
                return tab

            # block 0 + wq feed the very first matmuls: stream them k16-fine
            # on SEPARATE issue queues (one DMA engine per 128KB chunk) so
            # the PE can start as soon as chunk 0 of both has landed
            wq_sb = cpool.tile([128, NK, 4 * HD], XW)
            xt0 = xtpool.tile([128, NK, TQB], XW, name="xt0", tag="xt")
            for k16 in range(NK):
                nc.sync.dma_start(xt0[:, k16, :], xp[:, 0, k16, :])
                nc.scalar.dma_start(wq_sb[:, k16, :], wq[:, k16, :])
            tab0 = load_tab(0)

            # ---- cold constants on the scalar queue (parallel issue) ----
            wk_sb = cpool.tile([128, NK, HD], XW)
            nc.scalar.dma_start(wk_sb[:], wk[:])
            wv_sb = cpool.tile([128, NK, HD], XW)
            nc.scalar.dma_start(wv_sb[:], wv[:])
            Ef_sb = cpool.tile([128, 4, 4], MMD)
            nc.scalar.dma_start(Ef_sb[:], E4f[:])
            onesc_sb = cpool.tile([128, 1], PDT)
            nc.scalar.dma_start(onesc_sb[:], onesc[:])
            eye_sb = cpool.tile([128, 128], PDT)
            nc.scalar.dma_start(eye_sb[:], eye[:])
            E_sb = cpool.tile([128, 4, 4], PDT)
            nc.scalar.dma_start(E_sb[:], E4[:])
            pat_sb = cpool.tile([128, 4, TQB], PDT)
            nc.scalar.dma_start(pat_sb[:], pat[:])
            wo_sb = cpool.tile([128, NK, TQB], GDT)
            nc.scalar.dma_start(wo_sb[:], wo[:])
            sel_sb = None
            if not bcast_gpsimd:
                sel_sb = cpool.tile([4, 4, 128], MMD)
                nc.scalar.dma_start(sel_sb[:], sel4[:])
            epsq_sb = cpool.tile([128, 1], F32)
            nc.vector.memset(epsq_sb[:], EPS)
            epsk_sb = cpool.tile([128, 1], F32)
            nc.vector.memset(epsk_sb[:], float(HD) * EPS)

            # ---- persistent per-core state ----
            kT_sb = kvpool.tile([128, T], QKD)          # roped k^T
            v_sb = kvpool.tile([128, NK, HD], PDT)      # natural v
            rinvk_sb = kvpool.tile([128, NK], F32)      # 1/(rms_k*sqrt(HD))

            def bcast(dst, src, h, nm):
                """dst[128, TQB] f32 = broadcast of src[h, :]. gpsimd
                partition_broadcast needs the source row at partition 0, so
                rows 1..3 are staged there via a tiny SBUF->SBUF DMA."""
                if h == 0:
                    nc.gpsimd.partition_broadcast(dst, src[0:1, :])
                else:
                    st = smpool.tile([1, TQB], F32, name=f"stg{nm}",
                                     tag="stg", bufs=4)
                    nc.sync.dma_start(st[:], src[h:h + 1, :])
                    nc.gpsimd.partition_broadcast(dst, st[:])

            def q_block(j, xt, tab):
                # RoPE is applied to the *unnormalized* projection; the
                # rmsnorm scale (constant per head+token) is multiplied in
                # afterwards. This frees each qp PSUM bank without waiting
                # for the full 4-head sum-of-squares, so 2 banks suffice.
                cq_t, sq_tt = tab[:HD, 0, :], tab[:HD, 1, :]
                ssq = psC.tile([4, TQB], F32, name=f"ssq{j}", tag="psC")
                qs = []
                for pair in range(2):
                    qp = [psA.tile([128, TQB], F32, name=f"qp{j}_{pair}_{i}",
                                   tag="psA") for i in range(2)]
                    for k16 in range(NK):
                        for i in range(2):
                            h = 2 * pair + i
                            nc.tensor.matmul(
                                qp[i][:], wq_sb[:, k16, HD * h:HD * (h + 1)],
                                xt[:, k16, :],
                                start=(k16 == 0), stop=(k16 == NK - 1))
                    for i in range(2):
                        h = 2 * pair + i
                        s = wpool.tile([128, TQB], MMD, name=f"sqh{j}_{h}",
                                       tag="sqh", bufs=1)
                        nc.scalar.square(s[:], qp[i][:])
                        nc.tensor.matmul(ssq[:], Ef_sb[:, h, :], s[:],
                                         start=(h == 0), stop=(h == 3))
                        rot = wpool.tile([128, TQB], QKD, name=f"rot{j}_{h}",
                                         tag="rot", bufs=1)
                        nc.scalar.activation(rot[0:64, :], qp[i][64:128, :],
                                             AF.Copy, scale=-1.0)
                        nc.scalar.copy(rot[64:128, :], qp[i][0:64, :])
                        m1 = wpool.tile([128, TQB], QKD, name=f"m1{j}_{h}",
                                        tag="m1")
                        nc.vector.tensor_mul(m1[:], qp[i][:], cq_t)
                        m2 = wpool.tile([128, TQB], QKD, name=f"m2{j}_{h}",
                                        tag="m2")
                        nc.vector.tensor_mul(m2[:], rot[:], sq_tt)
                        qr = wpool.tile([128, TQB], QKD, name=f"qr{j}_{h}",
                                        tag="qr", bufs=4)
                        nc.vector.tensor_add(qr[:], m1[:], m2[:])
                        qs.append(qr)
                rms = smpool.tile([4, TQB], F32, name=f"rms{j}", tag="rms",
                                  bufs=1)
                nc.scalar.activation(rms[:], ssq[:], AF.Sqrt,
                                     bias=epsq_sb[0:4, :], scale=1.0 / HD)
                rinvq = smpool.tile([4, TQB], F32, name=f"rinvq{j}",
                                    tag="rinvq", bufs=1)
                nc.vector.reciprocal_approx_fast(rinvq[:], rms[:])
                qT = qtpool.tile([128, 4, TQB], QKD, name=f"qT{j}", tag="qT")
                for h in range(4):
                    bcs = wpool.tile([128, TQB], F32, name=f"bcs{j}_{h}",
                                     tag="bcs", bufs=2)
                    bcast(bcs[:], rinvq, h, f"q{j}_{h}")
                    nc.vector.scalar_tensor_tensor(qT[:, h, :], qs[h][:], 1.0,
                                                   bcs[:], OP.mult, OP.mult)
                return qT

            def kv_block(j, xt, tab):
                ck_t, sk_tt = tab[:HD, 2, :], tab[:HD, 3, :]
                kp = psC.tile([128, TQB], F32, name=f"kp{j}", tag="psC")
                for k16 in range(NK):
                    nc.tensor.matmul(kp[:], wk_sb[:, k16, :], xt[:, k16, :],
                                     start=(k16 == 0), stop=(k16 == NK - 1))
                sqk = wpool.tile([128, TQB], BF16, name=f"sqk{j}", tag="sqk",
                                 bufs=1)
                nc.scalar.square(sqk[:], kp[:])
                kssq = psC.tile([128, 4], F32, name=f"kssq{j}", tag="psC")
                for u in range(4):
                    nc.tensor.matmul(kssq[:, u:u + 1],
                                     sqk[:, 128 * u:128 * (u + 1)], onesc_sb[:],
                                     start=True, stop=True)
                rmsk = smpool.tile([128, 4], F32, name=f"rmsk{j}", tag="rmsk",
                                   bufs=1)
                nc.scalar.activation(rmsk[:], kssq[:], AF.Sqrt,
                                     bias=epsk_sb[:], scale=1.0)
                nc.vector.reciprocal_approx_fast(
                    rinvk_sb[:, 4 * j:4 * (j + 1)], rmsk[:])
                rotk = wpool.tile([128, TQB], F32, name=f"rotk{j}", tag="rot2",
                                  bufs=1)
                nc.scalar.activation(rotk[0:64, :], kp[64:128, :], AF.Copy,
                                     scale=-1.0)
                nc.scalar.copy(rotk[64:128, :], kp[0:64, :])
                m1k = wpool.tile([128, TQB], F32, name=f"m1k{j}", tag="m1")
                nc.vector.tensor_mul(m1k[:], kp[:], ck_t)
                m2k = wpool.tile([128, TQB], F32, name=f"m2k{j}", tag="m2")
                nc.vector.tensor_mul(m2k[:], rotk[:], sk_tt)
                nc.vector.tensor_add(kT_sb[:, TQB * j:TQB * (j + 1)],
                                     m1k[:], m2k[:])
                vp = psC.tile([128, TQB], F32, name=f"vp{j}", tag="psC")
                for k16 in range(NK):
                    nc.tensor.matmul(vp[:], wv_sb[:, k16, :], xt[:, k16, :],
                                     start=(k16 == 0), stop=(k16 == NK - 1))
                vT_t = wpool.tile([128, TQB], PDT, name=f"vT{j}", tag="vT",
                                  bufs=1)
                nc.vector.tensor_copy(vT_t[:], vp[:])
                vn = psA.tile([128, TQB], PDT, name=f"vn{j}", tag="psA")
                for u in range(4):
                    nc.tensor.transpose(vn[:, 128 * u:128 * (u + 1)],
                                        vT_t[:, 128 * u:128 * (u + 1)],
                                        eye_sb[:])
                for u in range(4):
                    nc.vector.tensor_copy(v_sb[:, 4 * j + u, :],
                                          vn[:, 128 * u:128 * (u + 1)])

            def attn_block(j, qT, n_g, diag_blk):
                ot = [psOT.tile([128, TQB], F32, name=f"ot{j}_{h}", tag="psOT")
                      for h in range(4)]
                lsum = smpool.tile([4, TQB], F32, name=f"lsum{j}", tag="lsum",
                                   bufs=1)
                for g in range(n_g):
                    diag = (g // 4 == diag_blk)
                    pts = []
                    for h in range(4):
                        sps = psC.tile([128, TQB], F32, name=f"s{j}_{g}_{h}",
                                       tag="psC")
                        nc.tensor.matmul(sps[:],
                                         kT_sb[:, 128 * g:128 * (g + 1)],
                                         qT[:, h, :], start=True, stop=True)
                        p_t = ppool.tile([128, TQB], PDT, name=f"p{j}_{g}_{h}",
                                         tag="p")
                        nc.scalar.activation(p_t[:], sps[:], AF.Exp,
                                             scale=rinvk_sb[:, g:g + 1])
                        if diag:
                            nc.vector.tensor_mul(p_t[:], p_t[:],
                                                 pat_sb[:, g % 4, :])
                        pts.append(p_t)
                    lg = psC.tile([4, TQB], F32, name=f"l{j}_{g}", tag="psC")
                    for h in range(4):
                        nc.tensor.matmul(lg[:], E_sb[:, h, :], pts[h][:],
                                         start=(h == 0), stop=(h == 3))
                        nc.tensor.matmul(ot[h][:], v_sb[:, g, :], pts[h][:],
                                         start=(g == 0), stop=(g == n_g - 1))
                    if g == 0:
                        nc.vector.tensor_copy(lsum[:], lg[:])
                    else:
                        nc.vector.tensor_add(lsum[:], lsum[:], lg[:])
                linv = smpool.tile([4, TQB], F32, name=f"linv{j}", tag="linv",
                                   bufs=1)
                nc.vector.reciprocal_approx_fast(linv[:], lsum[:])
                ags = []
                for half in range(2):
                    ag_in = dpool.tile([2 * HD, TQB], GDT,
                                       name=f"agin{j}_{half}", tag="agin")
                    for hh in range(2):
                        h = 2 * half + hh
                        bcs = wpool.tile([128, TQB], F32,
                                         name=f"bcso{j}_{h}", tag="bcs",
                                         bufs=2)
                        bcast(bcs[:], linv, h, f"o{j}_{h}")
                        on = w2pool.tile([128, TQB], GDT, name=f"on{j}_{h}",
                                         tag="on")
                        nc.vector.scalar_tensor_tensor(on[:], ot[h][:], 1.0,
                                                       bcs[:], OP.mult,
                                                       OP.mult)
                        nc.sync.dma_start(ag_in[128 * hh:128 * (hh + 1), :],
                                          on[:])
                    ag_out = dpool.tile([4 * 2 * HD, TQB], GDT,
                                        name=f"agout{j}_{half}", tag="agout")
                    if single:
                        for rr in range(4):
                            nc.sync.dma_start(
                                ag_out[256 * rr:256 * (rr + 1), :], ag_in[:])
                    else:
                        nc.gpsimd.collective_compute(
                            "AllGather", OP.bypass, replica_groups=GROUPS,
                            ins=[ag_in.opt()], outs=[ag_out.opt()])
                    ags.append(ag_out)
                return ags

            def wo_block(jj, ags):
                # token strips in pairs so fin needs only 2 PSUM banks;
                # og slabs carry 4 c16 slices per DMA
                for sh in range(2):
                    fins = [psA.tile([128, TQB], F32,
                                     name=f"fin{jj}_{sh}_{tt}", tag="psA")
                            for tt in range(2)]
                    cnt = 0
                    for half, ag in enumerate(ags):
                        for q4 in range(2):
                            og_t = ogpool.tile(
                                [128, 4, 256], GDT,
                                name=f"og{jj}_{sh}_{half}_{q4}", tag="og")
                            nc.sync.dma_start(
                                og_t[:],
                                ag[512 * q4:512 * (q4 + 1),
                                   256 * sh:256 * (sh + 1)]
                                .rearrange("(f p) c -> p f c", p=128))
                            for fi in range(4):
                                c16 = C16_HALF[half][4 * q4 + fi]
                                for tt in range(2):
                                    nc.tensor.matmul(
                                        fins[tt][:],
                                        og_t[:, fi, 128 * tt:128 * (tt + 1)],
                                        wo_sb[:, c16, :],
                                        start=(cnt == 0), stop=(cnt == 15))
                                cnt += 1
                    for tt in range(2):
                        t = 2 * sh + tt
                        fin_sb = smpool.tile([128, TQB], F32,
                                             name=f"finsb{jj}_{t}",
                                             tag="finsb")
                        nc.vector.tensor_copy(fin_sb[:], fins[tt][:])
                        nc.sync.dma_start(out[TQB * jj + 128 * t:
                                              TQB * jj + 128 * (t + 1), :],
                                          fin_sb[:])

            def wo_block_last(jj, ags):
                # final block: attn is done, so the 4 ot PSUM banks are
                # free -- use them for 4 fin strips, og loaded full width,
                # AG halves outer so half-a matmuls overlap AG half-b
                fins = [psOT.tile([128, TQB], F32, name=f"finL_{tt}",
                                  tag="psOT") for tt in range(4)]
                cnt = 0
                for half, ag in enumerate(ags):
                    for q4 in range(2):
                        og_t = ogpool.tile([128, 4, TQB], GDT,
                                           name=f"ogL_{half}_{q4}",
                                           tag="ogL", bufs=2)
                        nc.sync.dma_start(
                            og_t[:],
                            ag[512 * q4:512 * (q4 + 1), :]
                            .rearrange("(f p) c -> p f c", p=128))
                        for fi in range(4):
                            c16 = C16_HALF[half][4 * q4 + fi]
                            for tt in range(4):
                                nc.tensor.matmul(
                                    fins[tt][:],
                                    og_t[:, fi, 128 * tt:128 * (tt + 1)],
                                    wo_sb[:, c16, :],
                                    start=(cnt == 0), stop=(cnt == 15))
                            cnt += 1
                fin_sb = smpool.tile([128, 4, TQB], F32, name="finsbL",
                                     tag="finsbL", bufs=1)
                for tt in range(4):
                    nc.vector.tensor_copy(fin_sb[:, tt, :], fins[tt][:])
                nc.sync.dma_start(
                    out[TQB * jj:TQB * (jj + 1), :]
                    .rearrange("(f p) c -> p f c", p=128),
                    fin_sb[:])

            fin_prev = None
            xt, tab = xt0, tab0
            if causal:
                for j in range(NB):
                    nxt = (load_block(j + 1), load_tab(j + 1)) \
                        if j + 1 < NB else (None, None)
                    qT = q_block(j, xt, tab)
                    kv_block(j, xt, tab)
                    ags = attn_block(j, qT, 4 * (j + 1), j)
                    if fin_prev is not None:
                        wo_block(*fin_prev)
                    fin_prev = (j, ags)
                    xt, tab = nxt
                wo_block_last(*fin_prev)
            else:
                for j in range(NB):
                    nxt = (load_block(j + 1), load_tab(j + 1)) \
                        if j + 1 < NB else (None, None)
                    q_kv_xt = xt
                    kv_block(j, q_kv_xt, tab)
                    xt, tab = nxt
                # reload x for q pass
                xtq, tabq = load_block(0), load_tab(0)
                for j in range(NB):
                    nxt = (load_block(j + 1), load_tab(j + 1)) \
                        if j + 1 < NB else (None, None)
                    qT = q_block(j, xtq, tabq)
                    ags = attn_block(j, qT, 4 * NB, -1)
                    if fin_prev is not None:
                        wo_block(*fin_prev)
                    fin_prev = (j, ags)
                    xtq, tabq = nxt
                wo_block_last(*fin_prev)

    nc.compile()
    return nc


# ---------------- host-side prep ----------------

def _perm():
    return np.concatenate([np.arange(0, HD, 2), np.arange(1, HD, 2)])


def prep_core_inputs(x, Wq, Wk, Wv, Wo, q_scale, k_scale, cos, sin,
                     p_dt_bf16=True, g_dt_bf16=True, qk_bf16=True):
    import ml_dtypes
    pdt = ml_dtypes.bfloat16
    gdt = ml_dtypes.bfloat16
    qkd = ml_dtypes.bfloat16 if qk_bf16 else np.float32

    perm = _perm()
    partner = np.concatenate([np.arange(64, 128), np.arange(0, 64)])

    cosP = np.ascontiguousarray(cos[:, perm].T)   # [HD, T]
    sinP = np.ascontiguousarray(sin[:, perm].T)
    qsP, ksP = q_scale[perm], k_scale[perm]
    cq = (cosP * qsP[:, None]).astype(np.float32)
    sq = (sinP * qsP[partner][:, None]).astype(np.float32)
    ck = (cosP * ksP[:, None]).astype(np.float32)
    sk = (sinP * ksP[partner][:, None]).astype(np.float32)
    # tabs[p, b, t4, c] ; table order cq, sq, ck, sk
    tabs = np.stack([cq, sq, ck, sk], axis=1)        # [HD, 4, T]
    tabs = tabs.reshape(HD, 4, NB, TQB).transpose(0, 2, 1, 3)
    tabs = np.ascontiguousarray(tabs).astype(qkd)    # [128, NB, 4, TQB]

    patv = np.zeros((4, 128, TQB), np.float32)
    for u in range(4):
        tk = 128 * u + np.arange(128)
        patv[u] = (tk[:, None] <= np.arange(TQB)[None, :]).astype(np.float32)
    patp = np.ascontiguousarray(patv.transpose(1, 0, 2)).astype(pdt)
    E4 = np.zeros((4, 128, 4), np.float32)
    for h in range(4):
        E4[h][:, h] = 1.0
    Ep = np.ascontiguousarray(E4.transpose(1, 0, 2))   # [128, 4, 4]
    sel4 = np.zeros((4, 4, 128), np.float32)
    for h in range(4):
        sel4[h][h, :] = 1.0
    onesc = np.ones((128, 1), np.float32)
    eye = np.eye(128, dtype=np.float32).astype(pdt)

    def rearr_w(W, cols, n):
        # -> [128, NK, n] with [p, k, i] = W[128k+p, cols[i]]
        Wc = np.asarray(W[:, cols], np.float32)
        return np.ascontiguousarray(
            Wc.reshape(NK, 128, n).transpose(1, 0, 2))

    xps = []
    for b in range(B):
        xT = np.asarray(x[b], np.float32).T            # [D, T]
        xp = xT.reshape(NK, 128, NB, TQB).transpose(1, 2, 0, 3)
        xps.append(np.ascontiguousarray(xp).astype(qkd))

    in_maps = []
    for c in range(N_CORES):
        b, r = c // 4, c % 4
        wq_cols = np.concatenate([(4 * r + h) * HD + perm for h in range(4)])
        in_maps.append({
            "xp": xps[b],
            "wq": rearr_w(Wq, wq_cols, 4 * HD).astype(qkd),
            "wk": rearr_w(Wk, r * HD + perm, HD).astype(qkd),
            "wv": rearr_w(Wv, np.arange(r * HD, (r + 1) * HD), HD).astype(qkd),
            "wo": rearr_w(Wo, np.arange(r * TQB, (r + 1) * TQB),
                          TQB).astype(gdt),
            "tabs": tabs,
            "pat": patp,
            "E4": Ep.astype(pdt), "E4f": Ep.astype(np.float32),
            "sel4": sel4, "onesc": onesc.astype(pdt), "eye": eye,
        })
    return in_maps


def assemble_output(results):
    out = np.empty((B, T, D), np.float32)
    for c in range(N_CORES):
        b, r = c // 4, c % 4
        out[b][:, r * TQB:(r + 1) * TQB] = results[c]["out"]
    return out

_NC_CACHE = {}

P16, G16, QK16 = True, True, True


def _get_nc(causal=True):
    key = causal
    if key not in _NC_CACHE:
        _NC_CACHE[key] = build(mm_fast=True, p_dt_bf16=P16, g_dt_bf16=G16,
                               qk_bf16=QK16, causal=causal)
    return _NC_CACHE[key]


def kernel(x, Wq, Wk, Wv, Wo, q_scale, k_scale, cos, sin, mask):
    x = np.asarray(x, np.float32)
    Wq = np.asarray(Wq, np.float32); Wk = np.asarray(Wk, np.float32)
    Wv = np.asarray(Wv, np.float32); Wo = np.asarray(Wo, np.float32)
    q_scale = np.asarray(q_scale, np.float32)
    k_scale = np.asarray(k_scale, np.float32)
    cos = np.asarray(cos, np.float32); sin = np.asarray(sin, np.float32)
    m = np.asarray(mask).reshape(T, T)

    causal = bool(np.array_equal(m, np.tril(np.ones((T, T), bool))))
    if not causal and not m.all():
        return _host_reference(x, Wq, Wk, Wv, Wo, q_scale, k_scale, cos,
                               sin, np.asarray(mask))

    nc = _get_nc(causal=causal)
    in_maps = prep_core_inputs(x, Wq, Wk, Wv, Wo, q_scale, k_scale,
                               cos, sin, p_dt_bf16=P16, g_dt_bf16=G16,
                               qk_bf16=QK16)
    res = bass_utils.run_bass_kernel_spmd(nc, in_maps,
                                          core_ids=list(range(N_CORES)))
    return assemble_output(res.results)


def _host_reference(x, Wq, Wk, Wv, Wo, q_scale, k_scale, cos, sin, mask):
    # correctness fallback for masks that are neither causal nor all-true
    def rms(v, s):
        var = np.mean(np.square(v), axis=-1, keepdims=True)
        return v / np.sqrt(var + EPS) * s

    def rope(v, c, s):
        vr = np.stack([-v[..., 1::2], v[..., 0::2]], axis=-1)
        vr = vr.reshape(*vr.shape[:-2], -1)
        return v * c[None, :, None, :] + vr * s[None, :, None, :]

    q = (x @ Wq).reshape(B, T, H, HD)
    k = (x @ Wk).reshape(B, T, KV, HD)
    v = (x @ Wv).reshape(B, T, KV, HD)
    q = rope(rms(q, q_scale), cos, sin)
    k = rope(rms(k, k_scale), cos, sin)
    k = np.repeat(k, H // KV, axis=2)
    v = np.repeat(v, H // KV, axis=2)
    sc = np.einsum("bqhd,bkhd->bhqk", q, k) / np.sqrt(np.float32(HD))
    sc = np.where(np.asarray(mask).reshape(1, 1, T, T), sc, np.float32(-3.4e38))
    sc = sc - sc.max(axis=-1, keepdims=True)
    e = np.exp(sc)
    attn = e / e.sum(axis=-1, keepdims=True)
    o = np.einsum("bhqk,bkhd->bqhd", attn, v).reshape(B, T, H * HD)
    return (o @ Wo).astype(np.float32)
